# revision 20
# baseline (speedup 1.0000x reference)
"""Single-head attention (B=8, S=2048, D=U=1024) on 8 TRN2 NeuronCores.

Sharding: data-parallel over batch — core b computes batch b end-to-end,
no cross-core communication.

Per-core pipeline (fp32 PSUM accumulation everywhere):
  A. x [S,D] f32 arrives as 16 x 128-row blocks alternating between the
     sync and scalar HWDGE rings, is DVE-cast to bf16, then PE-transposed
     (128x128 tiles, identity rhs, bf16 PSUM) into xT [D,S] bf16.  No
     DRAM bounce and no serialized xbar: transposes ride the tensor
     queue, block-group-wise, one group ahead of the Q projections.
  B. W halves load as 1MB f32->bf16 SWDGE cast quarters on the gpsimd
     queue in consumption order; Wq.0's descriptors precede even the
     transpose identities since its transfer gates the first Q group.
  C. Qt = Wq^T xT + bq [U,S], Kt likewise, V = xT^T Wv + bv [S,U].
     Qt/Kt u-tiles 0-5 are stored fp8e4 (TRN e4m3), u-tiles 6-7 bf16 —
     see D.  No 1/sqrt(U) fold here; it moves into the exp scale.
     Mask/bias constants avoid slow single-partition/strided DMAs:
     bq/bk load as [16,128] and the mask as [16,128], then PE-transpose
     into per-partition column layouts.  m_row/bv_row single-partition
     loads and the m/bv broadcast matmuls sit after Q half 0 so their
     high DMA latency stays off the critical path.
  D. scores^T[k,q] = sum_u Kt[u,k] Qt[u,q]: u-tiles 0-5 via fp8
     DoubleRow matmuls (K=256/instr, ~1.8x bf16 throughput), u-tiles
     6-7 in bf16.  The 6/8-fp8 hybrid keeps the end-to-end rel err at
     1.58e-2 (vs 1.81e-2 all-fp8, 2e-2 budget) while capturing most of
     the fp8 speedup.  The padding mask adds the rank-1 term c_k*m_q
     (c = -320000*(1-m), pre-scaled by 32) via one DVE
     scalar_tensor_tensor per PSUM tile; Et = exp(scores^T/32) on ACT
     (scale=1/32 folded into the activation), PSUM->SBUF bf16.  No
     max-subtraction: scores/32 are O(1) and masked entries underflow
     to exactly 0, matching the fp32 reference.
  E. ctx[q,u] = sum_k Et[k,q]^T V[k,u]  (lhsT=Et -> natural output
     layout); denom[q] via extra N=1 ones-column matmuls under the same
     stationary Et (~18ns each, effectively free); out = ctx*(1/denom)
     in the epilogue, stored and DMA'd as bf16 (host upcasts to f32).
"""

import os
import sys

import numpy as np

for _p in ("/opt/trn_rl_repo", "/opt/pypackages"):
    if _p not in sys.path and os.path.isdir(_p):
        sys.path.append(_p)

import concourse.bass as bass
import concourse.tile as tile
from concourse import bacc, mybir
from concourse.bass import ts
from concourse.bass_utils import run_bass_kernel_spmd
from concourse.masks import make_identity

P = 128
B, S, D, U = 8, 2048, 1024, 1024
NCORES = 8
NG = 512  # matmul moving free dim (one fp32 PSUM bank)
DT, UT, ST, KT = D // P, U // P, S // P, S // P  # 8, 8, 16, 16
SG, QG = S // NG, S // NG  # 4, 4
UG = U // NG  # 2
UH = UT // 2  # u-tiles per W half
XB = S // P  # 16 x staging blocks of 128 rows
SCALE = 1.0 / 32.0  # 1/sqrt(U)

F32 = mybir.dt.float32
BF16 = mybir.dt.bfloat16
FP8 = mybir.dt.float8e4
I32 = mybir.dt.int32
AF = mybir.ActivationFunctionType
ALU = mybir.AluOpType
DR = mybir.MatmulPerfMode.DoubleRow

_cache = {}
last_results = None


def _emit(tc):
    nc = tc.nc
    x_d = nc.dram_tensor("x", [S, D], F32, kind="ExternalInput").ap()
    m_d = nc.dram_tensor("mask", [1, S], I32, kind="ExternalInput").ap()
    w_d = {
        "q": nc.dram_tensor("wq", [D, U], F32, kind="ExternalInput").ap(),
        "k": nc.dram_tensor("wk", [D, U], F32, kind="ExternalInput").ap(),
        "v": nc.dram_tensor("wv", [D, U], F32, kind="ExternalInput").ap(),
    }
    bq_d = nc.dram_tensor("bq", [1, U], F32, kind="ExternalInput").ap()
    bk_d = nc.dram_tensor("bk", [1, U], F32, kind="ExternalInput").ap()
    bv_d = nc.dram_tensor("bv", [1, U], F32, kind="ExternalInput").ap()
    out_d = nc.dram_tensor("out", [S, U], BF16, kind="ExternalOutput").ap()

    # ---------------- small persistent tensors ----------------
    # consts: [bq_cols 8][bk_cols 8][c_cols 16]
    consts, free_consts = tc.tile(shape=[P, 2 * UT + KT], dtype=F32, name="consts")
    bqk_cols = consts[:, 0 : 2 * UT]
    bq_cols = consts[:, 0:UT]
    bk_cols = consts[:, UT : 2 * UT]
    c_cols = consts[:, 2 * UT : 2 * UT + KT]  # -10000*(1-m), per k partition

    rows, free_rows = tc.tile(shape=[1, S + U + P], dtype=BF16, name="rows")
    m_row = rows[:, 0:S]
    bv_row = rows[:, S : S + U]
    ones_row = rows[:, S + U : S + U + P]

    ones_col, free_ones_col = tc.tile(shape=[P, 1], dtype=BF16, name="ones_col")
    m_bcast, free_m_bcast = tc.tile(shape=[P, S], dtype=BF16, name="m_bcast")
    bv_bcast, free_bv_bcast = tc.tile(shape=[P, U], dtype=BF16, name="bv_bcast")
    idf, free_idf = tc.tile(shape=[16, 16], dtype=F32, name="idf")
    idb, free_idb = tc.tile(shape=[P, P], dtype=BF16, name="idb")
    m16b, free_m16b = tc.tile(shape=[16, P], dtype=BF16, name="m16b")

    with tc.tile_pool(name="big", bufs=1) as big:

        def load_w_half(which, half, pieces=2):
            wt = big.tile([P, DT, NG], BF16, tag="w", bufs=2, name=f"w{which}_{half}")
            src = w_d[which].rearrange("(t p) u -> p t u", p=P)
            w = NG // pieces
            for q in range(pieces):
                # f32 -> bf16 cast (SWDGE), NG/pieces cols per transfer
                nc.gpsimd.dma_start(
                    wt[:, :, ts(q, w)], src[:, :, half * NG + q * w : half * NG + (q + 1) * w]
                )
            return wt

        # Wq.0's SWDGE descriptors go first in the gpsimd queue (its
        # transfer gates the first Q group), then the PE-transpose
        # identities (gpsimd-only affine_select), then the other W halves.
        wq_h0 = load_w_half("q", 0, pieces=4)
        make_identity(nc, idf)
        make_identity(nc, idb)

        # small multi-partition const loads first on the sync ring
        # (contiguous per partition; must not trail the big x transfers)
        bqk8 = big.tile([2 * UT, P], F32, tag="qt", name="bqk8")
        nc.sync.dma_start(bqk8[0:UT, :], bq_d.rearrange("a (j p) -> j (a p)", j=UT))
        nc.sync.dma_start(
            bqk8[UT : 2 * UT, :], bk_d.rearrange("a (j p) -> j (a p)", j=UT)
        )
        m16 = big.tile([KT, P], I32, tag="kt", name="m16")
        nc.sync.dma_start(m16[:], m_d.rearrange("a (t p) -> t (a p)", t=KT))
        nc.vector.tensor_copy(m16b[:], m16[:])

        nc.vector.memset(ones_row, 1.0)
        nc.vector.memset(ones_col[:], 1.0)

        # --- gpsimd/SWDGE queue: remaining W halves, consumption order ---
        wq_h1 = load_w_half("q", 1)
        wk_h = [load_w_half("k", 0), load_w_half("k", 1)]
        wv_h = [load_w_half("v", 0), load_w_half("v", 1)]

        # --- x blocks alternate between the scalar and sync HWDGE rings ---
        xstg = []
        for xb in range(XB):
            eng, tag = (nc.scalar, "stgL") if xb % 2 == 0 else (nc.sync, "stgS")
            stg = big.tile([P, D], F32, tag=tag, bufs=2, name=f"x_{xb}")
            eng.dma_start(stg[:], x_d[ts(xb, P), :])
            xstg.append(stg)

        # single-partition loads: slow (many us latency) but consumed late
        m_i32 = big.tile([1, S], I32, tag="kt", name="m_i32")
        nc.sync.dma_start(m_i32[:], m_d)
        bv_f32 = big.tile([1, U], F32, tag="v", name="bv_f32")
        nc.sync.dma_start(bv_f32[:], bv_d)

        # ---------------- phases A+C ----------------
        # slotA holds xT (A-C) then Et (D-E); sized for Et (64KB/partition).
        xT = big.tile([P, DT, S], BF16, tag="slotA", name="xT")
        qt8_sb = big.tile([P, 6, S], FP8, tag="qt", name="qt8_sb")
        qtb_sb = big.tile([P, 2, S], BF16, tag="qtb", name="qtb_sb")
        kt8_sb = big.tile([P, 6, S], FP8, tag="kt", name="kt8_sb")
        ktb_sb = big.tile([P, 2, S], BF16, tag="ktb", name="ktb_sb")
        v_sb = big.tile([P, ST, U], BF16, tag="v", name="v_sb")

        with (
            tc.tile_pool(name="psA", bufs=2, space="PSUM") as psA,
            tc.tile_pool(name="psC", bufs=4, space="PSUM") as psC,
        ):
            # bq/bk and mask to per-partition column layout via PE transpose
            pb = psA.tile([P, NG], F32, tag="tp", name="ps_bqk")
            nc.tensor.transpose(
                pb[:, 0 : 2 * UT], bqk8[:], idf[:]
            )
            nc.vector.tensor_copy(bqk_cols, pb[:, 0 : 2 * UT])
            pm = psA.tile([P, NG], BF16, tag="tp", name="ps_m16")
            nc.tensor.transpose(pm[:, 0:KT], m16b[:], idb[0:KT, 0:KT])
            # c = m*10000 - 10000  -> 0 where m==1, -10000 where m==0
            nc.vector.tensor_scalar(
                c_cols, pm[:, 0:KT], 320000.0, -320000.0, ALU.mult, ALU.add
            )

            def proj_group(dst8, dstb, bias_cols, sg, half, w_h):
                for u4 in range(UH):
                    ut = half * UH + u4
                    ps = psC.tile([P, NG], F32, tag="proj", name="ps_proj")
                    for dt in range(DT):
                        nc.tensor.matmul(
                            ps[:],
                            lhsT=w_h[:, dt, ts(u4, P)],
                            rhs=xT[:, dt, ts(sg, NG)],
                            start=(dt == 0),
                            stop=(dt == DT - 1),
                        )
                    dst, j = (dst8, ut) if ut < 6 else (dstb, ut - 6)
                    nc.vector.tensor_scalar_add(
                        dst[:, j, ts(sg, NG)], ps[:], bias_cols[:, ut : ut + 1]
                    )

            # transposes of x block xb (128 rows -> one column group of xT);
            # blocks are DVE-cast to bf16 first so the PE transposes run at
            # 1 cycle/row.  Casts for group sg+1 are emitted before group
            # sg's Q epilogues so the vector queue never gates the PE.
            xbf = []

            def cast_block(xb):
                stb = big.tile([P, D], BF16, tag="stgB", bufs=4, name=f"xb_{xb}")
                nc.vector.tensor_copy(stb[:], xstg[xb][:])
                xbf.append(stb)

            def transpose_block(xb):
                for half in range(2):
                    pt = psA.tile([P, NG], BF16, tag="tp", name="ps_tp")
                    for d4 in range(4):
                        dt = half * 4 + d4
                        nc.tensor.transpose(
                            pt[:, ts(d4, P)], xbf[xb][:, ts(dt, P)], idb[:]
                        )
                    dst = xT[:, ts(half, 4), ts(xb, P)]
                    src = pt[:].rearrange("p (a b) -> p a b", a=4)
                    nc.vector.tensor_copy(dst, src)

            for j in range(8):
                cast_block(j)
            for j in range(8):
                transpose_block(j)
            for sg in range(SG):
                if sg >= 1 and sg < SG - 1:
                    for j in range(4):
                        cast_block(4 * (sg + 1) + j)
                    for j in range(4):
                        transpose_block(4 * (sg + 1) + j)
                proj_group(qt8_sb, qtb_sb, bq_cols, sg, 0, wq_h0)

            # m/bv row casts + broadcast matmuls, off the critical path now
            nc.vector.tensor_copy(m_row, m_i32[:])
            nc.vector.tensor_copy(bv_row, bv_f32[:])
            for qg in range(QG):
                pi = psC.tile([P, NG], F32, tag="proj", name="ps_init")
                nc.tensor.matmul(
                    pi[:], lhsT=ones_row[:, 0:P], rhs=m_row[:, ts(qg, NG)]
                )
                nc.vector.tensor_copy(m_bcast[:, ts(qg, NG)], pi[:])
            for ug in range(UG):
                pi = psC.tile([P, NG], F32, tag="proj", name="ps_init2")
                nc.tensor.matmul(
                    pi[:], lhsT=ones_row[:, 0:P], rhs=bv_row[:, ts(ug, NG)]
                )
                nc.vector.tensor_copy(bv_bcast[:, ts(ug, NG)], pi[:])

            for sg in range(SG):
                proj_group(qt8_sb, qtb_sb, bq_cols, sg, 1, wq_h1)

            # K^T
            for half in range(2):
                for sg in range(SG):
                    proj_group(kt8_sb, ktb_sb, bk_cols, sg, half, wk_h[half])

            # V: [s,u] = sum_d xT[d,s] * Wv[d,u]; bv added in the epilogue
            for ug in range(UG):
                for st in range(ST):
                    pv = psC.tile([P, NG], F32, tag="proj", name="ps_v")
                    for dt in range(DT):
                        nc.tensor.matmul(
                            pv[:],
                            lhsT=xT[:, dt, ts(st, P)],
                            rhs=wv_h[ug][:, dt, :],
                            start=(dt == 0),
                            stop=(dt == DT - 1),
                        )
                    nc.vector.tensor_tensor(
                        v_sb[:, st, ts(ug, NG)],
                        pv[:],
                        bv_bcast[:, ts(ug, NG)],
                        ALU.add,
                    )

        # ---------------- phase D: scores^T + mask + exp ----------------
        et_sb = big.tile([P, KT, S], BF16, tag="slotA", name="et_sb")
        with tc.tile_pool(name="psD", bufs=4, space="PSUM") as psD:
            for kt in range(KT):
                pss = [
                    psD.tile([P, NG], F32, tag="sc", name="ps_sc") for _ in range(QG)
                ]
                for j in range(3):
                    lhsT = kt8_sb[:, 2 * j : 2 * j + 2, ts(kt, P)]
                    for qg in range(QG):
                        nc.tensor.matmul(
                            pss[qg][:],
                            lhsT=lhsT,
                            rhs=qt8_sb[:, 2 * j : 2 * j + 2, ts(qg, NG)],
                            start=(j == 0),
                            stop=False,
                            perf_mode=DR,
                        )
                for jb in range(2):
                    lhsT = ktb_sb[:, jb, ts(kt, P)]
                    for qg in range(QG):
                        nc.tensor.matmul(
                            pss[qg][:],
                            lhsT=lhsT,
                            rhs=qtb_sb[:, jb, ts(qg, NG)],
                            start=False,
                            stop=(jb == 1),
                        )
                for qg in range(QG):
                    # scores += c_k * m_q  (rank-1 mask term, on DVE)
                    nc.vector.scalar_tensor_tensor(
                        pss[qg][:],
                        m_bcast[:, ts(qg, NG)],
                        c_cols[:, kt : kt + 1],
                        pss[qg][:],
                        ALU.mult,
                        ALU.add,
                    )
                    nc.scalar.activation(
                        et_sb[:, kt, ts(qg, NG)], pss[qg][:], AF.Exp, scale=SCALE
                    )

        # ---------------- phase E: PV + denom + normalize ----------------
        with (
            tc.tile_pool(name="psE", bufs=4, space="PSUM") as psE,
            tc.tile_pool(name="psDen", bufs=2, space="PSUM") as psDen,
        ):
            for qt in range(KT):
                pc = [
                    psE.tile([P, NG], F32, tag="ctx", name="ps_ctx")
                    for _ in range(UG)
                ]
                den = psDen.tile([P, 1], F32, tag="den", name="ps_den")
                for kt in range(KT):
                    lhsT = et_sb[:, kt, ts(qt, P)]
                    first, last = kt == 0, kt == KT - 1
                    for ug in range(UG):
                        nc.tensor.matmul(
                            pc[ug][:],
                            lhsT=lhsT,
                            rhs=v_sb[:, kt, ts(ug, NG)],
                            start=first,
                            stop=last,
                        )
                    nc.tensor.matmul(
                        den[:], lhsT=lhsT, rhs=ones_col[:], start=first, stop=last
                    )
                recip = big.tile([P, 1], F32, tag="kt", name="recip")
                nc.vector.reciprocal(recip[:], den[:])
                o = big.tile([P, U], BF16, tag="qt", name="o_sb")
                for ug in range(UG):
                    nc.vector.tensor_scalar_mul(o[:, ts(ug, NG)], pc[ug][:], recip[:])
                nc.sync.dma_start(out_d[ts(qt, P), :], o[:])

    free_m16b()
    free_idb()
    free_idf()
    free_bv_bcast()
    free_m_bcast()
    free_ones_col()
    free_rows()
    free_consts()


def _build():
    if "nc" in _cache:
        return _cache["nc"]
    nc = bacc.Bacc("TRN2", target_bir_lowering=False, debug=False, num_devices=NCORES)
    with tile.TileContext(nc) as tc:
        _emit(tc)
    nc.compile()
    _cache["nc"] = nc
    return nc


def kernel(x, mask, Wq, bq, Wk, bk, Wv, bv):
    global last_results
    nc = _build()
    wq = np.ascontiguousarray(Wq, dtype=np.float32)
    wk = np.ascontiguousarray(Wk, dtype=np.float32)
    wv = np.ascontiguousarray(Wv, dtype=np.float32)
    bqr = np.ascontiguousarray(bq, dtype=np.float32).reshape(1, U)
    bkr = np.ascontiguousarray(bk, dtype=np.float32).reshape(1, U)
    bvr = np.ascontiguousarray(bv, dtype=np.float32).reshape(1, U)
    in_maps = []
    for b in range(B):
        in_maps.append(
            {
                "x": np.ascontiguousarray(x[b], dtype=np.float32),
                "mask": np.ascontiguousarray(mask[b], dtype=np.int32).reshape(1, S),
                "wq": wq,
                "wk": wk,
                "wv": wv,
                "bq": bqr,
                "bk": bkr,
                "bv": bvr,
            }
        )
    res = run_bass_kernel_spmd(
        nc,
        in_maps,
        core_ids=list(range(NCORES)),
        trace=bool(int(os.environ.get("KERNEL_TRACE", "0"))),
        tmpdir=os.environ.get("KERNEL_TRACE_DIR"),
    )
    last_results = res
    return np.stack([res.results[b]["out"].astype(np.float32) for b in range(B)])


# revision 21
# speedup vs baseline: 1.0131x; 1.0131x over previous
"""Single-head attention (B=8, S=2048, D=U=1024) on 8 TRN2 NeuronCores.

Sharding: data-parallel over batch — core b computes batch b end-to-end,
no cross-core communication.

Per-core pipeline (fp32 PSUM accumulation everywhere):
  A. x [S,D] f32 arrives as 16 x 128-row blocks alternating between the
     sync and scalar HWDGE rings, is DVE-cast to bf16, then PE-transposed
     (128x128 tiles, identity rhs, bf16 PSUM) into xT [D,S] bf16.  No
     DRAM bounce and no serialized xbar: transposes ride the tensor
     queue, block-group-wise, one group ahead of the Q projections.
  B. W halves load as 1MB f32->bf16 SWDGE cast quarters on the gpsimd
     queue in consumption order; Wq.0's descriptors precede even the
     transpose identities since its transfer gates the first Q group.
  C. Qt = Wq^T xT + bq [U,S], Kt likewise, V = xT^T Wv + bv [S,U].
     Qt/Kt u-tiles 0-5 are stored fp8e4 (TRN e4m3), u-tiles 6-7 bf16 —
     see D.  No 1/sqrt(U) fold here; it moves into the exp scale.
     Mask/bias constants avoid slow single-partition/strided DMAs:
     bq/bk load as [16,128] and the mask as [16,128], then PE-transpose
     into per-partition column layouts.  m_row/bv_row single-partition
     loads and the m/bv broadcast matmuls sit after Q half 0 so their
     high DMA latency stays off the critical path.
  D. scores^T[k,q] = sum_u Kt[u,k] Qt[u,q]: u-tiles 0-5 via fp8
     DoubleRow matmuls (K=256/instr, ~1.8x bf16 throughput), u-tiles
     6-7 in bf16.  The 6/8-fp8 hybrid keeps the end-to-end rel err at
     1.58e-2 (vs 1.81e-2 all-fp8, 2e-2 budget) while capturing most of
     the fp8 speedup.  The padding mask adds the rank-1 term c_k*m_q
     (c = -320000*(1-m), pre-scaled by 32) via one DVE
     scalar_tensor_tensor per PSUM tile; Et = exp(scores^T/32) on ACT
     (scale=1/32 folded into the activation), PSUM->SBUF bf16.  No
     max-subtraction: scores/32 are O(1) and masked entries underflow
     to exactly 0, matching the fp32 reference.
  E. ctx[q,u] = sum_k Et[k,q]^T V[k,u]  (lhsT=Et -> natural output
     layout); denom[q] via extra N=1 ones-column matmuls under the same
     stationary Et (~18ns each, effectively free); out = ctx*(1/denom)
     in the epilogue, stored and DMA'd as bf16 (host upcasts to f32).
"""

import os
import sys

import numpy as np

for _p in ("/opt/trn_rl_repo", "/opt/pypackages"):
    if _p not in sys.path and os.path.isdir(_p):
        sys.path.append(_p)

import concourse.bass as bass
import concourse.tile as tile
from concourse import bacc, mybir
from concourse.bass import ts
from concourse.bass_utils import run_bass_kernel_spmd
from concourse.masks import make_identity

P = 128
B, S, D, U = 8, 2048, 1024, 1024
NCORES = 8
NG = 512  # matmul moving free dim (one fp32 PSUM bank)
DT, UT, ST, KT = D // P, U // P, S // P, S // P  # 8, 8, 16, 16
SG, QG = S // NG, S // NG  # 4, 4
UG = U // NG  # 2
UH = UT // 2  # u-tiles per W half
XB = S // P  # 16 x staging blocks of 128 rows
SCALE = 1.0 / 32.0  # 1/sqrt(U)

F32 = mybir.dt.float32
BF16 = mybir.dt.bfloat16
FP8 = mybir.dt.float8e4
I32 = mybir.dt.int32
AF = mybir.ActivationFunctionType
ALU = mybir.AluOpType
DR = mybir.MatmulPerfMode.DoubleRow

_cache = {}
last_results = None


def _emit(tc):
    nc = tc.nc
    x_d = nc.dram_tensor("x", [S, D], F32, kind="ExternalInput").ap()
    m_d = nc.dram_tensor("mask", [1, S], I32, kind="ExternalInput").ap()
    w_d = {
        "q": nc.dram_tensor("wq", [D, U], F32, kind="ExternalInput").ap(),
        "k": nc.dram_tensor("wk", [D, U], F32, kind="ExternalInput").ap(),
        "v": nc.dram_tensor("wv", [D, U], F32, kind="ExternalInput").ap(),
    }
    bq_d = nc.dram_tensor("bq", [1, U], F32, kind="ExternalInput").ap()
    bk_d = nc.dram_tensor("bk", [1, U], F32, kind="ExternalInput").ap()
    bv_d = nc.dram_tensor("bv", [1, U], F32, kind="ExternalInput").ap()
    out_d = nc.dram_tensor("out", [S, U], BF16, kind="ExternalOutput").ap()

    # ---------------- small persistent tensors ----------------
    # consts: [bq_cols 8][bk_cols 8][c_cols 16]
    consts, free_consts = tc.tile(shape=[P, 2 * UT + KT], dtype=F32, name="consts")
    bqk_cols = consts[:, 0 : 2 * UT]
    bq_cols = consts[:, 0:UT]
    bk_cols = consts[:, UT : 2 * UT]
    c_cols = consts[:, 2 * UT : 2 * UT + KT]  # -10000*(1-m), per k partition

    rows, free_rows = tc.tile(shape=[1, S + U + P], dtype=BF16, name="rows")
    m_row = rows[:, 0:S]
    bv_row = rows[:, S : S + U]
    ones_row = rows[:, S + U : S + U + P]

    ones_col, free_ones_col = tc.tile(shape=[P, 1], dtype=BF16, name="ones_col")
    m_bcast, free_m_bcast = tc.tile(shape=[P, S], dtype=BF16, name="m_bcast")
    bv_bcast, free_bv_bcast = tc.tile(shape=[P, U], dtype=BF16, name="bv_bcast")
    idf, free_idf = tc.tile(shape=[16, 16], dtype=F32, name="idf")
    idb, free_idb = tc.tile(shape=[P, P], dtype=BF16, name="idb")
    m16b, free_m16b = tc.tile(shape=[16, P], dtype=BF16, name="m16b")

    with tc.tile_pool(name="big", bufs=1) as big:

        def load_w_half(which, half):
            wt = big.tile([P, DT, NG], BF16, tag="w", bufs=2, name=f"w{which}_{half}")
            src = w_d[which].rearrange("(t p) u -> p t u", p=P)
            for q in range(2):
                uq = half * NG + q * (NG // 2)
                # f32 -> bf16 cast (SWDGE), 1MB per transfer
                nc.gpsimd.dma_start(
                    wt[:, :, ts(q, NG // 2)], src[:, :, uq : uq + NG // 2]
                )
            return wt

        # Wq.0's SWDGE descriptors go first in the gpsimd queue (its
        # transfer gates the first Q group), then the PE-transpose
        # identities (gpsimd-only affine_select), then the other W halves.
        wq_h0 = load_w_half("q", 0)
        make_identity(nc, idf)
        make_identity(nc, idb)

        # small multi-partition const loads first on the sync ring
        # (contiguous per partition; must not trail the big x transfers)
        bqk8 = big.tile([2 * UT, P], F32, tag="qt", name="bqk8")
        nc.sync.dma_start(bqk8[0:UT, :], bq_d.rearrange("a (j p) -> j (a p)", j=UT))
        nc.sync.dma_start(
            bqk8[UT : 2 * UT, :], bk_d.rearrange("a (j p) -> j (a p)", j=UT)
        )
        m16 = big.tile([KT, P], I32, tag="kt", name="m16")
        nc.sync.dma_start(m16[:], m_d.rearrange("a (t p) -> t (a p)", t=KT))
        nc.vector.tensor_copy(m16b[:], m16[:])

        nc.vector.memset(ones_row, 1.0)
        nc.vector.memset(ones_col[:], 1.0)

        # --- gpsimd/SWDGE queue: remaining W halves, consumption order ---
        wq_h1 = load_w_half("q", 1)
        wk_h = [load_w_half("k", 0), load_w_half("k", 1)]
        wv_h = [load_w_half("v", 0), load_w_half("v", 1)]

        # --- x blocks alternate between the scalar and sync HWDGE rings ---
        xstg = []
        for xb in range(XB):
            eng, tag = (nc.scalar, "stgL") if xb % 2 == 0 else (nc.sync, "stgS")
            stg = big.tile([P, D], F32, tag=tag, bufs=2, name=f"x_{xb}")
            eng.dma_start(stg[:], x_d[ts(xb, P), :])
            xstg.append(stg)

        # single-partition loads: slow (many us latency) but consumed late
        m_i32 = big.tile([1, S], I32, tag="kt", name="m_i32")
        nc.sync.dma_start(m_i32[:], m_d)
        bv_f32 = big.tile([1, U], F32, tag="v", name="bv_f32")
        nc.sync.dma_start(bv_f32[:], bv_d)

        # ---------------- phases A+C ----------------
        # slotA holds xT (A-C) then Et (D-E); sized for Et (64KB/partition).
        xT = big.tile([P, DT, S], BF16, tag="slotA", name="xT")
        qt8_sb = big.tile([P, 6, S], FP8, tag="qt", name="qt8_sb")
        qtb_sb = big.tile([P, 2, S], BF16, tag="qtb", name="qtb_sb")
        kt8_sb = big.tile([P, 6, S], FP8, tag="kt", name="kt8_sb")
        ktb_sb = big.tile([P, 2, S], BF16, tag="ktb", name="ktb_sb")
        v_sb = big.tile([P, ST, U], BF16, tag="v", name="v_sb")

        with (
            tc.tile_pool(name="psA", bufs=2, space="PSUM") as psA,
            tc.tile_pool(name="psC", bufs=6, space="PSUM") as psC,
        ):
            # bq/bk and mask to per-partition column layout via PE transpose
            pb = psA.tile([P, NG], F32, tag="tp", name="ps_bqk")
            nc.tensor.transpose(
                pb[:, 0 : 2 * UT], bqk8[:], idf[:]
            )
            nc.vector.tensor_copy(bqk_cols, pb[:, 0 : 2 * UT])
            pm = psA.tile([P, NG], BF16, tag="tp", name="ps_m16")
            nc.tensor.transpose(pm[:, 0:KT], m16b[:], idb[0:KT, 0:KT])
            # c = m*10000 - 10000  -> 0 where m==1, -10000 where m==0
            nc.vector.tensor_scalar(
                c_cols, pm[:, 0:KT], 320000.0, -320000.0, ALU.mult, ALU.add
            )

            def proj_group(dst8, dstb, bias_cols, sg, half, w_h):
                for u4 in range(UH):
                    ut = half * UH + u4
                    ps = psC.tile([P, NG], F32, tag="proj", name="ps_proj")
                    for dt in range(DT):
                        nc.tensor.matmul(
                            ps[:],
                            lhsT=w_h[:, dt, ts(u4, P)],
                            rhs=xT[:, dt, ts(sg, NG)],
                            start=(dt == 0),
                            stop=(dt == DT - 1),
                        )
                    dst, j = (dst8, ut) if ut < 6 else (dstb, ut - 6)
                    nc.vector.tensor_scalar_add(
                        dst[:, j, ts(sg, NG)], ps[:], bias_cols[:, ut : ut + 1]
                    )

            # transposes of x block xb (128 rows -> one column group of xT);
            # blocks are DVE-cast to bf16 first so the PE transposes run at
            # 1 cycle/row.  Casts for group sg+1 are emitted before group
            # sg's Q epilogues so the vector queue never gates the PE.
            xbf = []

            def cast_block(xb):
                stb = big.tile([P, D], BF16, tag="stgB", bufs=4, name=f"xb_{xb}")
                nc.vector.tensor_copy(stb[:], xstg[xb][:])
                xbf.append(stb)

            def transpose_block(xb):
                for half in range(2):
                    pt = psA.tile([P, NG], BF16, tag="tp", name="ps_tp")
                    for d4 in range(4):
                        dt = half * 4 + d4
                        nc.tensor.transpose(
                            pt[:, ts(d4, P)], xbf[xb][:, ts(dt, P)], idb[:]
                        )
                    dst = xT[:, ts(half, 4), ts(xb, P)]
                    src = pt[:].rearrange("p (a b) -> p a b", a=4)
                    nc.vector.tensor_copy(dst, src)

            for j in range(8):
                cast_block(j)
            for j in range(8):
                transpose_block(j)
            for sg in range(SG):
                if sg >= 1 and sg < SG - 1:
                    for j in range(4):
                        cast_block(4 * (sg + 1) + j)
                    for j in range(4):
                        transpose_block(4 * (sg + 1) + j)
                proj_group(qt8_sb, qtb_sb, bq_cols, sg, 0, wq_h0)

            # m/bv row casts + broadcast matmuls, off the critical path now
            nc.vector.tensor_copy(m_row, m_i32[:])
            nc.vector.tensor_copy(bv_row, bv_f32[:])
            for qg in range(QG):
                pi = psC.tile([P, NG], F32, tag="proj", name="ps_init")
                nc.tensor.matmul(
                    pi[:], lhsT=ones_row[:, 0:P], rhs=m_row[:, ts(qg, NG)]
                )
                nc.vector.tensor_copy(m_bcast[:, ts(qg, NG)], pi[:])
            for ug in range(UG):
                pi = psC.tile([P, NG], F32, tag="proj", name="ps_init2")
                nc.tensor.matmul(
                    pi[:], lhsT=ones_row[:, 0:P], rhs=bv_row[:, ts(ug, NG)]
                )
                nc.vector.tensor_copy(bv_bcast[:, ts(ug, NG)], pi[:])

            for sg in range(SG):
                proj_group(qt8_sb, qtb_sb, bq_cols, sg, 1, wq_h1)

            # K^T
            for half in range(2):
                for sg in range(SG):
                    proj_group(kt8_sb, ktb_sb, bk_cols, sg, half, wk_h[half])

            # V: [s,u] = sum_d xT[d,s] * Wv[d,u]; bv added in the epilogue
            for ug in range(UG):
                for st in range(ST):
                    pv = psC.tile([P, NG], F32, tag="proj", name="ps_v")
                    for dt in range(DT):
                        nc.tensor.matmul(
                            pv[:],
                            lhsT=xT[:, dt, ts(st, P)],
                            rhs=wv_h[ug][:, dt, :],
                            start=(dt == 0),
                            stop=(dt == DT - 1),
                        )
                    nc.vector.tensor_tensor(
                        v_sb[:, st, ts(ug, NG)],
                        pv[:],
                        bv_bcast[:, ts(ug, NG)],
                        ALU.add,
                    )

        # ---------------- phase D: scores^T + mask + exp ----------------
        et_sb = big.tile([P, KT, S], BF16, tag="slotA", name="et_sb")
        with tc.tile_pool(name="psD", bufs=4, space="PSUM") as psD:
            for kt in range(KT):
                pss = [
                    psD.tile([P, NG], F32, tag="sc", name="ps_sc") for _ in range(QG)
                ]
                for j in range(3):
                    lhsT = kt8_sb[:, 2 * j : 2 * j + 2, ts(kt, P)]
                    for qg in range(QG):
                        nc.tensor.matmul(
                            pss[qg][:],
                            lhsT=lhsT,
                            rhs=qt8_sb[:, 2 * j : 2 * j + 2, ts(qg, NG)],
                            start=(j == 0),
                            stop=False,
                            perf_mode=DR,
                        )
                for jb in range(2):
                    lhsT = ktb_sb[:, jb, ts(kt, P)]
                    for qg in range(QG):
                        nc.tensor.matmul(
                            pss[qg][:],
                            lhsT=lhsT,
                            rhs=qtb_sb[:, jb, ts(qg, NG)],
                            start=False,
                            stop=(jb == 1),
                        )
                for qg in range(QG):
                    # scores += c_k * m_q  (rank-1 mask term, on DVE)
                    nc.vector.scalar_tensor_tensor(
                        pss[qg][:],
                        m_bcast[:, ts(qg, NG)],
                        c_cols[:, kt : kt + 1],
                        pss[qg][:],
                        ALU.mult,
                        ALU.add,
                    )
                    nc.scalar.activation(
                        et_sb[:, kt, ts(qg, NG)], pss[qg][:], AF.Exp, scale=SCALE
                    )

        # ---------------- phase E: PV + denom + normalize ----------------
        with (
            tc.tile_pool(name="psE", bufs=4, space="PSUM") as psE,
            tc.tile_pool(name="psDen", bufs=2, space="PSUM") as psDen,
        ):
            for qt in range(KT):
                pc = [
                    psE.tile([P, NG], F32, tag="ctx", name="ps_ctx")
                    for _ in range(UG)
                ]
                den = psDen.tile([P, 1], F32, tag="den", name="ps_den")
                for kt in range(KT):
                    lhsT = et_sb[:, kt, ts(qt, P)]
                    first, last = kt == 0, kt == KT - 1
                    for ug in range(UG):
                        nc.tensor.matmul(
                            pc[ug][:],
                            lhsT=lhsT,
                            rhs=v_sb[:, kt, ts(ug, NG)],
                            start=first,
                            stop=last,
                        )
                    nc.tensor.matmul(
                        den[:], lhsT=lhsT, rhs=ones_col[:], start=first, stop=last
                    )
                recip = big.tile([P, 1], F32, tag="kt", name="recip")
                nc.vector.reciprocal(recip[:], den[:])
                o = big.tile([P, U], BF16, tag="qt", name="o_sb")
                for ug in range(UG):
                    nc.vector.tensor_scalar_mul(o[:, ts(ug, NG)], pc[ug][:], recip[:])
                nc.sync.dma_start(out_d[ts(qt, P), :], o[:])

    free_m16b()
    free_idb()
    free_idf()
    free_bv_bcast()
    free_m_bcast()
    free_ones_col()
    free_rows()
    free_consts()


def _build():
    if "nc" in _cache:
        return _cache["nc"]
    nc = bacc.Bacc("TRN2", target_bir_lowering=False, debug=False, num_devices=NCORES)
    with tile.TileContext(nc) as tc:
        _emit(tc)
    nc.compile()
    _cache["nc"] = nc
    return nc


def kernel(x, mask, Wq, bq, Wk, bk, Wv, bv):
    global last_results
    nc = _build()
    wq = np.ascontiguousarray(Wq, dtype=np.float32)
    wk = np.ascontiguousarray(Wk, dtype=np.float32)
    wv = np.ascontiguousarray(Wv, dtype=np.float32)
    bqr = np.ascontiguousarray(bq, dtype=np.float32).reshape(1, U)
    bkr = np.ascontiguousarray(bk, dtype=np.float32).reshape(1, U)
    bvr = np.ascontiguousarray(bv, dtype=np.float32).reshape(1, U)
    in_maps = []
    for b in range(B):
        in_maps.append(
            {
                "x": np.ascontiguousarray(x[b], dtype=np.float32),
                "mask": np.ascontiguousarray(mask[b], dtype=np.int32).reshape(1, S),
                "wq": wq,
                "wk": wk,
                "wv": wv,
                "bq": bqr,
                "bk": bkr,
                "bv": bvr,
            }
        )
    res = run_bass_kernel_spmd(
        nc,
        in_maps,
        core_ids=list(range(NCORES)),
        trace=bool(int(os.environ.get("KERNEL_TRACE", "0"))),
        tmpdir=os.environ.get("KERNEL_TRACE_DIR"),
    )
    last_results = res
    return np.stack([res.results[b]["out"].astype(np.float32) for b in range(B)])
